# revision 30
# baseline (speedup 1.0000x reference)
"""Causal multi-head attention (B=4, S=2048, D=1024, H=16) on 8 TRN2 NeuronCores.

Sharding: 4 batches x 2 head-groups (8 heads each) -> 8 cores.
Each core:
  - projects its batch's tokens through its head-group's Wq/Wk/Wv columns in
    transposed [head_dim, token] layout (no on-device transposes); q/k are
    stored in a [64, hl, pair, token] layout so both heads of a pair sit at
    base partition 0 (no staging copies before the 64-contraction matmuls),
  - computes causal attention (mask = tril(k=1): one future token allowed)
    for its 8 heads; scoresT blocks [k,q] are exponentiated on the scalar
    engine and multiplied by {0,1} masks on the vector engine. Score matmul +
    exp skip the fully-masked column range near the diagonal; the skipped et
    columns are memset to 0. Softmax denominators come from a ones-column
    appended to V so the PV matmul accumulates both ctx^T and the exp-sums.
    Normalization is deferred: raw ctx^T and the sums are staged to SBUF, a
    batched reciprocal_approx_fast + DMA-broadcast applies 1/sum per token
    half just before the output projection.
  - computes the partial output projection ctx_part @ Wo[group rows] + bo/2
    per token half; ReduceScatter(add, bf16) per half overlaps the second
    half's compute. The host casts bf16 back to f32 and concatenates.

All matmuls run in bf16 (PSUM accumulates fp32); projections are interleaved
with attention so the tensor engine never drains between phases.
"""

import numpy as np

B, S, D = 4, 2048, 1024
H = 16
HD = D // H  # 64
G = 2  # head groups (tensor-parallel degree per batch)
HPG = H // G  # 8 heads per core
DG = D // G  # 512 dims per group
P = 128
NKT = D // P  # 8 k-tiles over d_model
NQC = S // 512  # 4 query chunks of 512
NTT = S // P  # 16 token tiles of 128
NR = DG // P  # 4 dim-tiles (head pairs) per group
SH = S // 2  # tokens per RS half (per core pair)

_CACHE = {}


def _build_masks():
    """masks[s] is the [128, 512] multiplicative mask for a scoresT block
    [k_local, q_chunk_local] whose k-block index is kb = 4*qc + s.
    Allowed iff global k <= global q + 1."""
    masks = np.zeros((5, P, 512), dtype=np.float32)
    i = np.arange(P)[:, None]  # k local
    jj = np.arange(P)[None, :]  # q local within 128-subblock
    for s in range(5):
        for j in range(4):  # q subblock within the 512 chunk
            blk = masks[s][:, 128 * j : 128 * (j + 1)]
            if j > s:
                blk[:] = 1.0
            elif j == s:
                blk[:] = (i <= jj + 1).astype(np.float32)
            elif j == s - 1:
                blk[0, 127] = 1.0
    return masks


def _build_bass():
    import concourse.bacc as bacc
    import concourse.mybir as mybir
    import concourse.tile as tile

    f32 = mybir.dt.float32
    bf16 = mybir.dt.bfloat16
    AF = mybir.ActivationFunctionType

    nc = bacc.Bacc("TRN2", target_bir_lowering=False, debug=False, num_devices=8)

    xT = nc.dram_tensor("xT", [D, S], bf16, kind="ExternalInput").ap()
    wq = nc.dram_tensor("wq", [D, DG], bf16, kind="ExternalInput").ap()
    wk = nc.dram_tensor("wk", [D, DG], bf16, kind="ExternalInput").ap()
    wv = nc.dram_tensor("wv", [D, DG], bf16, kind="ExternalInput").ap()
    wo = nc.dram_tensor("wo", [DG, D], bf16, kind="ExternalInput").ap()
    bo_b = nc.dram_tensor("bo_b", [P, D], f32, kind="ExternalInput").ap()
    masks = nc.dram_tensor("masks", [5, P, 512], bf16, kind="ExternalInput").ap()
    out_ext = nc.dram_tensor("out", [S // 2, D], bf16, kind="ExternalOutput").ap()

    with tile.TileContext(nc) as tc:
        with (
            tc.tile_pool(name="pqk", bufs=1) as pqk,
            tc.tile_pool(name="pv", bufs=1) as pv,
            tc.tile_pool(name="pmask", bufs=1) as pmask,
            tc.tile_pool(name="pw", bufs=1) as pw,
            tc.tile_pool(name="px", bufs=2) as px,
            tc.tile_pool(name="pe", bufs=2) as pe,
            tc.tile_pool(name="pn", bufs=2) as pn,
            tc.tile_pool(name="po_sb", bufs=2) as po_sb,
            tc.tile_pool(name="psum_s", bufs=1) as psums,
            tc.tile_pool(name="pp", bufs=2, space="PSUM") as pp,
            tc.tile_pool(name="psS", bufs=2, space="PSUM") as psS,
            tc.tile_pool(name="psC", bufs=1, space="PSUM") as psC,
            tc.tile_pool(name="pdram", bufs=1, space="DRAM") as pdram,
        ):
            # persistent SBUF tensors
            qT_sb = pqk.tile([64, G, NR, S], bf16)  # [dims | hl, pair, token]
            kT_sb = pqk.tile([64, G, NR, S], bf16)
            va_sb = pv.tile([P, NTT, HPG, HD + 1], bf16)  # v + ones col
            ctxT_sb = pqk.tile([P, NR, S], bf16)  # raw ctx^T, normalized in place
            masks_sb = pmask.tile([P, 5, 512], bf16)
            # softmax denominators, parked on partition 0 (engine SBUF writes
            # must start on a partition quad): row = ((qc%2)*NR+pr)*G+hl,
            # reused across qc pairs (the broadcast read orders the reuse)
            sums_sb = psums.tile([1, 2 * NR * G, 512], f32)

            nc.sync.dma_start(masks_sb[:], masks.rearrange("s p q -> p s q"))
            # ones column of va: masks[s=0] block j=3 is all 1.0 (j > s), and
            # memset can't encode the immediate, so copy ones from there.
            nc.vector.tensor_copy(
                va_sb[:, :, :, HD : HD + 1],
                masks_sb[:, 0, 384:512].rearrange("p (a b) -> p a b", b=HPG)[
                    :, :, :, None
                ],
            )

            xT_r0 = xT.rearrange("(ko p) t -> p ko t", p=P)
            xt0 = px.tile([P, NKT, 512], bf16, name="xtile", tag="x")
            # weights (wq + x chunk 0 first, sliced by k-tile so the first
            # projection matmul starts as soon as its first slices land;
            # wo/bo are issued after project(1) since they're needed late)
            w_sbs = {"wq": pw.tile([P, NKT, DG], bf16, name="w_wq")}
            wq_r = wq.rearrange("(ko p) f -> p ko f", p=P)
            for kt in range(NKT):
                nc.sync.dma_start(w_sbs["wq"][:, kt, :], wq_r[:, kt, :])
                nc.sync.dma_start(xt0[:, kt, :], xT_r0[:, kt, 0:512])
            for name, w in (("wk", wk), ("wv", wv)):
                w_sb = pw.tile([P, NKT, DG], bf16, name=f"w_{name}")
                nc.sync.dma_start(w_sb[:], w.rearrange("(ko p) f -> p ko f", p=P))
                w_sbs[name] = w_sb
            wo_sb = pw.tile([P, NR, D], bf16)
            bo_sb = pw.tile([P, D], f32)

            partial = [pdram.tile([512, D], bf16, name=f"partial{q}") for q in range(4)]
            rs_out = [pdram.tile([256, D], bf16, name=f"rs{q}") for q in range(4)]
            sums_d = pdram.tile([2 * NR * G, 512], f32)

            xT_r = xT.rearrange("(ko p) t -> p ko t", p=P)

            def project(t):
                tok = slice(512 * t, 512 * (t + 1))
                if t == 0:
                    xtile = xt0
                else:
                    xtile = px.tile([P, NKT, 512], bf16, name="xtile", tag="x")
                    nc.sync.dma_start(xtile[:], xT_r[:, :, tok])
                # qT / kT: out [dims(pair rr), 512 tokens], split by head
                for name, dst in (("wq", qT_sb), ("wk", kT_sb)):
                    w_sb = w_sbs[name]
                    for rr in range(NR):
                        ps = pp.tile([P, 512], f32, name="ps_proj", tag="ps")
                        for kt in range(NKT):
                            nc.tensor.matmul(
                                ps[:],
                                w_sb[:, kt, P * rr : P * (rr + 1)],
                                xtile[:, kt, :],
                                start=(kt == 0),
                                stop=(kt == NKT - 1),
                            )
                        nc.vector.tensor_copy(dst[:, 0, rr, tok], ps[0:64, :])
                        nc.vector.tensor_copy(dst[:, 1, rr, tok], ps[64:P, :])
                # v: out [128 tokens, 512 dims] per token tile
                w_sb = w_sbs["wv"]
                for st in range(4):
                    tt = 4 * t + st
                    ps = pp.tile([P, 512], f32, name="ps_v", tag="ps")
                    for kt in range(NKT):
                        nc.tensor.matmul(
                            ps[:],
                            xtile[:, kt, 128 * st : 128 * (st + 1)],
                            w_sb[:, kt, :],
                            start=(kt == 0),
                            stop=(kt == NKT - 1),
                        )
                    nc.vector.tensor_copy(
                        va_sb[:, tt, :, 0:HD],
                        ps[:].rearrange("p (h d) -> p h d", d=HD),
                    )

            def attend(qc, prs):
                qs = slice(512 * qc, 512 * (qc + 1))
                nkb = min(4 * qc + 5, NTT)
                for pr in prs:
                    ctxs = [
                        psC.tile([HD + 1, 512], f32, name=f"ctx{hl}", tag=f"ctx{hl}")
                        for hl in range(2)
                    ]
                    for kb in range(nkb):
                        ks = slice(128 * kb, 128 * (kb + 1))
                        s = kb - 4 * qc
                        masked = 0 <= s <= 4
                        # columns [0, c0) of this block are fully causally
                        # masked; skip them in the score matmul and exp, and
                        # memset the et range to zero for the PV matmul.
                        c0 = max(0, (s - 1) * 128) if masked else 0
                        sc = psS.tile([P, 1024], f32, name="sc", tag="sc")
                        et = pe.tile([P, 1024], bf16, name="et", tag="et")
                        for hl in range(2):
                            nc.tensor.matmul(
                                sc[:, 512 * hl + c0 : 512 * (hl + 1)],
                                kT_sb[:, hl, pr, ks],
                                qT_sb[:, hl, pr, 512 * qc + c0 : 512 * (qc + 1)],
                                start=True,
                                stop=True,
                            )
                        if c0 == 0:
                            # both heads' ranges are contiguous: one activation
                            nc.scalar.activation(
                                et[:], sc[:], AF.Exp, scale=1.0 / 8.0
                            )
                        else:
                            for hl in range(2):
                                nc.gpsimd.memset(et[:, 512 * hl : 512 * hl + c0], 0.0)
                                nc.scalar.activation(
                                    et[:, 512 * hl + c0 : 512 * (hl + 1)],
                                    sc[:, 512 * hl + c0 : 512 * (hl + 1)],
                                    AF.Exp,
                                    scale=1.0 / 8.0,
                                )
                        if masked:
                            c1 = min((s + 1) * 128, 512)
                            for hl in range(2):
                                nc.vector.tensor_mul(
                                    et[:, 512 * hl + c0 : 512 * hl + c1],
                                    et[:, 512 * hl + c0 : 512 * hl + c1],
                                    masks_sb[:, s, c0:c1],
                                )
                        for hl in range(2):
                            nc.tensor.matmul(
                                ctxs[hl][:],
                                va_sb[:, kb, 2 * pr + hl, :],
                                et[:, 512 * hl : 512 * (hl + 1)],
                                start=(kb == 0),
                                stop=(kb == nkb - 1),
                            )
                    # stage raw ctx + sums to SBUF; normalization is deferred
                    for hl in range(2):
                        row = ((qc % 2) * NR + pr) * G + hl
                        nc.vector.tensor_copy(
                            sums_sb[0:1, row, :], ctxs[hl][HD : HD + 1, :]
                        )
                        nc.vector.tensor_copy(
                            ctxT_sb[64 * hl : 64 * (hl + 1), pr, qs],
                            ctxs[hl][0:HD, :],
                        )

            def norm(qc, prs):
                # normalize ctxT for head-pairs `prs` of token chunk qc:
                # bounce the raw sums through DRAM, DMA-broadcast them into a
                # [128, 512] tile (both heads of a pair stacked), then one
                # in-place approx-reciprocal and one multiply per pair.
                r0 = ((qc % 2) * NR + prs[0]) * G
                nr = len(prs) * G
                nc.sync.dma_start(
                    sums_d[None, r0 : r0 + nr, :], sums_sb[0:1, r0 : r0 + nr, :]
                )
                qs = slice(512 * qc, 512 * (qc + 1))
                for pr in prs:
                    bc = pn.tile([P, 512], f32, name="bc", tag="bc")
                    for hl in range(2):
                        row = ((qc % 2) * NR + pr) * G + hl
                        nc.sync.dma_start(
                            bc[64 * hl : 64 * (hl + 1), :],
                            sums_d[row : row + 1, :].to_broadcast((64, 512)),
                        )
                    nc.vector.reciprocal_approx_fast(bc[:], bc[:])
                    sl = ctxT_sb[:, pr, qs]
                    nc.vector.tensor_mul(sl, sl, bc[:])

            def outproj_rs(qc, sts, rs_rows):
                # output projection for token tiles 4*qc+sts, then
                # reduce-scatter rows `rs_rows` of this chunk's partial with
                # the pair core (ordered after the tiles covering those rows).
                for st in sts:
                    tt = 4 * qc + st
                    ts_ = slice(128 * tt, 128 * (tt + 1))
                    td = slice(128 * st, 128 * (st + 1))
                    for nch in range(2):
                        ns = slice(512 * nch, 512 * (nch + 1))
                        ps = pp.tile([P, 512], f32, name="ps_o", tag="ps")
                        for rr in range(NR):
                            nc.tensor.matmul(
                                ps[:],
                                ctxT_sb[:, rr, ts_],
                                wo_sb[:, rr, ns],
                                start=(rr == 0),
                                stop=(rr == NR - 1),
                            )
                        ot = po_sb.tile([P, 512], bf16, name="ot", tag="ot")
                        nc.vector.tensor_add(ot[:], ps[:], bo_sb[:, ns])
                        nc.sync.dma_start(partial[qc][td, ns], ot[:])
                import concourse.mybir as mybir

                a, b = rs_rows
                nc.gpsimd.collective_compute(
                    "ReduceScatter",
                    mybir.AluOpType.add,
                    replica_groups=[[0, 1], [2, 3], [4, 5], [6, 7]],
                    ins=[partial[qc][a:b, :]],
                    outs=[rs_out[qc][a // 2 : b // 2, :]],
                )

            project(0)
            project(1)
            nc.sync.dma_start(wo_sb[:], wo.rearrange("(ko p) f -> p ko f", p=P))
            nc.sync.dma_start(bo_sb[:], bo_b[:])
            attend(0, [0, 1])
            attend(0, [2, 3])
            norm(0, [0, 1])
            norm(0, [2, 3])
            project(2)
            outproj_rs(0, [0, 1, 2, 3], (0, 512))
            attend(1, [0, 1])
            attend(1, [2, 3])
            norm(1, [0, 1])
            norm(1, [2, 3])
            project(3)
            outproj_rs(1, [0, 1, 2, 3], (0, 512))
            attend(2, [0, 1])
            attend(2, [2, 3])
            norm(2, [0, 1])
            norm(2, [2, 3])
            attend(3, [0, 1])
            outproj_rs(2, [0, 1, 2, 3], (0, 512))
            attend(3, [2])
            norm(3, [0, 1])
            attend(3, [3])
            norm(3, [2])
            norm(3, [3])
            outproj_rs(3, [0, 1, 2, 3], (0, 512))
            # rs_out -> out copies all sit at the end so their waits on the
            # collectives never head-of-line-block the DMA queue mid-kernel
            for qc in range(4):
                nc.sync.dma_start(
                    out_ext[256 * qc : 256 * (qc + 1), :], rs_out[qc][:]
                )

    nc.compile()
    return nc


def _in_maps(x, Wq, Wk, Wv, Wo, bo):
    import ml_dtypes

    bf16 = ml_dtypes.bfloat16
    masks = _build_masks().astype(bf16)
    maps = []
    for c in range(8):
        b, g = c // 2, c % 2
        cols = slice(DG * g, DG * (g + 1))
        maps.append(
            {
                "xT": np.ascontiguousarray(np.asarray(x)[b].T).astype(bf16),
                "wq": np.ascontiguousarray(np.asarray(Wq)[:, cols]).astype(bf16),
                "wk": np.ascontiguousarray(np.asarray(Wk)[:, cols]).astype(bf16),
                "wv": np.ascontiguousarray(np.asarray(Wv)[:, cols]).astype(bf16),
                "wo": np.ascontiguousarray(np.asarray(Wo)[cols, :]).astype(bf16),
                "bo_b": np.broadcast_to(
                    np.asarray(bo, dtype=np.float32) / G, (P, D)
                ).copy(),
                "masks": masks,
            }
        )
    return maps


def _get_nc():
    if "nc" not in _CACHE:
        _CACHE["nc"] = _build_bass()
    return _CACHE["nc"]


def run(inputs, trace=False):
    from concourse.bass_utils import run_bass_kernel_spmd

    nc = _get_nc()
    maps = _in_maps(**inputs)
    res = run_bass_kernel_spmd(nc, maps, list(range(8)), trace=trace)
    out = np.empty((B, S, D), dtype=np.float32)
    # reduce-scatter chunks as issued by the kernel: (token0, rows_in)
    chunks = [(0, 512), (512, 512), (1024, 512), (1536, 512)]
    for c in range(8):
        b, g = c // 2, c % 2
        r = np.asarray(res.results[c]["out"]).astype(np.float32)
        off = 0
        for tok0, L in chunks:
            h = L // 2
            out[b, tok0 + h * g : tok0 + h * (g + 1), :] = r[off : off + h]
            off += h
    return out, res


def kernel(x, Wq, Wk, Wv, Wo, bo):
    out, _ = run(dict(x=x, Wq=Wq, Wk=Wk, Wv=Wv, Wo=Wo, bo=bo))
    return out


# revision 33
# speedup vs baseline: 1.0986x; 1.0986x over previous
"""Causal multi-head attention (B=4, S=2048, D=1024, H=16) on 8 TRN2 NeuronCores.

Sharding: 4 batches x 2 head-groups (8 heads each) -> 8 cores.
Each core:
  - projects its batch's tokens through its head-group's Wq/Wk/Wv columns in
    transposed [head_dim, token] layout (no on-device transposes); q/k are
    stored in a [64, hl, pair, token] layout so both heads of a pair sit at
    base partition 0 (no staging copies before the 64-contraction matmuls),
  - computes causal attention (mask = tril(k=1): one future token allowed)
    for its 8 heads; scoresT blocks [k,q] are exponentiated on the scalar
    engine and multiplied by {0,1} masks on the vector engine. Score matmul +
    exp skip the fully-masked column range near the diagonal; the skipped et
    columns are memset to 0. Softmax denominators come from a ones-column
    appended to V so the PV matmul accumulates both ctx^T and the exp-sums.
    Normalization is deferred: raw ctx^T and the sums are staged to SBUF, a
    batched reciprocal_approx_fast + DMA-broadcast applies 1/sum per token
    half just before the output projection.
  - computes the partial output projection ctx_part @ Wo[group rows] + bo/2
    per token half; ReduceScatter(add, bf16) per half overlaps the second
    half's compute. The host casts bf16 back to f32 and concatenates.

All matmuls run in bf16 (PSUM accumulates fp32); projections are interleaved
with attention so the tensor engine never drains between phases.
"""

import numpy as np

B, S, D = 4, 2048, 1024
H = 16
HD = D // H  # 64
G = 2  # head groups (tensor-parallel degree per batch)
HPG = H // G  # 8 heads per core
DG = D // G  # 512 dims per group
P = 128
NKT = D // P  # 8 k-tiles over d_model
NQC = S // 512  # 4 query chunks of 512
NTT = S // P  # 16 token tiles of 128
NR = DG // P  # 4 dim-tiles (head pairs) per group
SH = S // 2  # tokens per RS half (per core pair)

_CACHE = {}


def _build_masks():
    """masks[s] is the [128, 512] multiplicative mask for a scoresT block
    [k_local, q_chunk_local] whose k-block index is kb = 4*qc + s.
    Allowed iff global k <= global q + 1."""
    masks = np.zeros((5, P, 512), dtype=np.float32)
    i = np.arange(P)[:, None]  # k local
    jj = np.arange(P)[None, :]  # q local within 128-subblock
    for s in range(5):
        for j in range(4):  # q subblock within the 512 chunk
            blk = masks[s][:, 128 * j : 128 * (j + 1)]
            if j > s:
                blk[:] = 1.0
            elif j == s:
                blk[:] = (i <= jj + 1).astype(np.float32)
            elif j == s - 1:
                blk[0, 127] = 1.0
    return masks


def _build_bass():
    import concourse.bacc as bacc
    import concourse.mybir as mybir
    import concourse.tile as tile

    f32 = mybir.dt.float32
    bf16 = mybir.dt.bfloat16
    AF = mybir.ActivationFunctionType

    nc = bacc.Bacc("TRN2", target_bir_lowering=False, debug=False, num_devices=8)

    xT = nc.dram_tensor("xT", [D, S], bf16, kind="ExternalInput").ap()
    wq = nc.dram_tensor("wq", [D, DG], bf16, kind="ExternalInput").ap()
    wk = nc.dram_tensor("wk", [D, DG], bf16, kind="ExternalInput").ap()
    wv = nc.dram_tensor("wv", [D, DG], bf16, kind="ExternalInput").ap()
    wo = nc.dram_tensor("wo", [DG, D], bf16, kind="ExternalInput").ap()
    bo_b = nc.dram_tensor("bo_b", [P, D], f32, kind="ExternalInput").ap()
    masks = nc.dram_tensor("masks", [5, P, 512], bf16, kind="ExternalInput").ap()
    out_ext = nc.dram_tensor("out", [S // 2, D], bf16, kind="ExternalOutput").ap()

    with tile.TileContext(nc) as tc:
        with (
            tc.tile_pool(name="pqk", bufs=1) as pqk,
            tc.tile_pool(name="pv", bufs=1) as pv,
            tc.tile_pool(name="pmask", bufs=1) as pmask,
            tc.tile_pool(name="pw", bufs=1) as pw,
            tc.tile_pool(name="px", bufs=2) as px,
            tc.tile_pool(name="pe", bufs=2) as pe,
            tc.tile_pool(name="pn", bufs=2) as pn,
            tc.tile_pool(name="po_sb", bufs=2) as po_sb,
            tc.tile_pool(name="psum_s", bufs=1) as psums,
            tc.tile_pool(name="pp", bufs=2, space="PSUM") as pp,
            tc.tile_pool(name="psS", bufs=2, space="PSUM") as psS,
            tc.tile_pool(name="psC", bufs=1, space="PSUM") as psC,
            tc.tile_pool(name="pdram", bufs=1, space="DRAM") as pdram,
        ):
            # persistent SBUF tensors
            qT_sb = pqk.tile([64, G, NR, S], bf16)  # [dims | hl, pair, token]
            kT_sb = pqk.tile([64, G, NR, S], bf16)
            va_sb = pv.tile([P, NTT, HPG, HD + 1], bf16)  # v + ones col
            ctxT_sb = pqk.tile([P, NR, S], bf16)  # raw ctx^T, normalized in place
            masks_sb = pmask.tile([P, 5, 512], bf16)
            # softmax denominators, parked on partition 0 (engine SBUF writes
            # must start on a partition quad): row = ((qc%2)*NR+pr)*G+hl,
            # reused across qc pairs (the broadcast read orders the reuse)
            sums_sb = psums.tile([1, 2 * NR * G, 512], f32)

            nc.sync.dma_start(masks_sb[:], masks.rearrange("s p q -> p s q"))
            # ones column of va: masks[s=0] block j=3 is all 1.0 (j > s), and
            # memset can't encode the immediate, so copy ones from there.
            nc.vector.tensor_copy(
                va_sb[:, :, :, HD : HD + 1],
                masks_sb[:, 0, 384:512].rearrange("p (a b) -> p a b", b=HPG)[
                    :, :, :, None
                ],
            )

            xT_r0 = xT.rearrange("(ko p) t -> p ko t", p=P)
            xt0 = px.tile([P, NKT, 512], bf16, name="xtile", tag="x")
            # weights (wq + x chunk 0 first, sliced by k-tile so the first
            # projection matmul starts as soon as its first slices land;
            # wo/bo are issued after project(1) since they're needed late)
            w_sbs = {"wq": pw.tile([P, NKT, DG], bf16, name="w_wq")}
            wq_r = wq.rearrange("(ko p) f -> p ko f", p=P)
            for kt in range(NKT):
                nc.sync.dma_start(w_sbs["wq"][:, kt, :], wq_r[:, kt, :])
                nc.sync.dma_start(xt0[:, kt, :], xT_r0[:, kt, 0:512])
            for name, w in (("wk", wk), ("wv", wv)):
                w_sb = pw.tile([P, NKT, DG], bf16, name=f"w_{name}")
                nc.sync.dma_start(w_sb[:], w.rearrange("(ko p) f -> p ko f", p=P))
                w_sbs[name] = w_sb
            wo_sb = pw.tile([P, NR, D], bf16)
            bo_sb = pw.tile([P, D], f32)

            partial = [pdram.tile([512, D], bf16, name=f"partial{q}") for q in range(4)]
            rs_out = [pdram.tile([256, D], bf16, name=f"rs{q}") for q in range(4)]

            xT_r = xT.rearrange("(ko p) t -> p ko t", p=P)

            def project(t):
                tok = slice(512 * t, 512 * (t + 1))
                if t == 0:
                    xtile = xt0
                else:
                    xtile = px.tile([P, NKT, 512], bf16, name="xtile", tag="x")
                    nc.sync.dma_start(xtile[:], xT_r[:, :, tok])
                # qT / kT: out [dims(pair rr), 512 tokens], split by head
                for name, dst in (("wq", qT_sb), ("wk", kT_sb)):
                    w_sb = w_sbs[name]
                    for rr in range(NR):
                        ps = pp.tile([P, 512], f32, name="ps_proj", tag="ps")
                        for kt in range(NKT):
                            nc.tensor.matmul(
                                ps[:],
                                w_sb[:, kt, P * rr : P * (rr + 1)],
                                xtile[:, kt, :],
                                start=(kt == 0),
                                stop=(kt == NKT - 1),
                            )
                        nc.vector.tensor_copy(dst[:, 0, rr, tok], ps[0:64, :])
                        nc.vector.tensor_copy(dst[:, 1, rr, tok], ps[64:P, :])
                # v: out [128 tokens, 512 dims] per token tile
                w_sb = w_sbs["wv"]
                for st in range(4):
                    tt = 4 * t + st
                    ps = pp.tile([P, 512], f32, name="ps_v", tag="ps")
                    for kt in range(NKT):
                        nc.tensor.matmul(
                            ps[:],
                            xtile[:, kt, 128 * st : 128 * (st + 1)],
                            w_sb[:, kt, :],
                            start=(kt == 0),
                            stop=(kt == NKT - 1),
                        )
                    nc.vector.tensor_copy(
                        va_sb[:, tt, :, 0:HD],
                        ps[:].rearrange("p (h d) -> p h d", d=HD),
                    )

            def attend(qc, prs):
                qs = slice(512 * qc, 512 * (qc + 1))
                nkb = min(4 * qc + 5, NTT)
                for pr in prs:
                    ctxs = [
                        psC.tile([HD + 1, 512], f32, name=f"ctx{hl}", tag=f"ctx{hl}")
                        for hl in range(2)
                    ]
                    for kb in range(nkb):
                        ks = slice(128 * kb, 128 * (kb + 1))
                        s = kb - 4 * qc
                        masked = 0 <= s <= 4
                        # columns [0, c0) of this block are fully causally
                        # masked; skip them in the score matmul and exp, and
                        # memset the et range to zero for the PV matmul.
                        c0 = max(0, (s - 1) * 128) if masked else 0
                        sc = psS.tile([P, 1024], f32, name="sc", tag="sc")
                        et = pe.tile([P, 1024], bf16, name="et", tag="et")
                        for hl in range(2):
                            nc.tensor.matmul(
                                sc[:, 512 * hl + c0 : 512 * (hl + 1)],
                                kT_sb[:, hl, pr, ks],
                                qT_sb[:, hl, pr, 512 * qc + c0 : 512 * (qc + 1)],
                                start=True,
                                stop=True,
                            )
                        if c0 == 0:
                            # both heads' ranges are contiguous: one activation
                            nc.scalar.activation(
                                et[:], sc[:], AF.Exp, scale=1.0 / 8.0
                            )
                        else:
                            for hl in range(2):
                                nc.gpsimd.memset(et[:, 512 * hl : 512 * hl + c0], 0.0)
                                nc.scalar.activation(
                                    et[:, 512 * hl + c0 : 512 * (hl + 1)],
                                    sc[:, 512 * hl + c0 : 512 * (hl + 1)],
                                    AF.Exp,
                                    scale=1.0 / 8.0,
                                )
                        if masked:
                            c1 = min((s + 1) * 128, 512)
                            for hl in range(2):
                                nc.vector.tensor_mul(
                                    et[:, 512 * hl + c0 : 512 * hl + c1],
                                    et[:, 512 * hl + c0 : 512 * hl + c1],
                                    masks_sb[:, s, c0:c1],
                                )
                        for hl in range(2):
                            nc.tensor.matmul(
                                ctxs[hl][:],
                                va_sb[:, kb, 2 * pr + hl, :],
                                et[:, 512 * hl : 512 * (hl + 1)],
                                start=(kb == 0),
                                stop=(kb == nkb - 1),
                            )
                    # stage raw ctx + sums to SBUF; normalization is deferred
                    for hl in range(2):
                        row = ((qc % 2) * NR + pr) * G + hl
                        nc.vector.tensor_copy(
                            sums_sb[0:1, row, :], ctxs[hl][HD : HD + 1, :]
                        )
                        nc.vector.tensor_copy(
                            ctxT_sb[64 * hl : 64 * (hl + 1), pr, qs],
                            ctxs[hl][0:HD, :],
                        )

            def norm(qc, prs):
                # normalize ctxT for head-pairs `prs` of token chunk qc: the
                # sums sit on partition 0, so a gpsimd partition-broadcast
                # fans them out (no DMA queues involved), then one in-place
                # approx-reciprocal and one multiply per pair.
                qs = slice(512 * qc, 512 * (qc + 1))
                for pr in prs:
                    row = ((qc % 2) * NR + pr) * G
                    bc = pn.tile([P, 512], f32, name="bc", tag="bc")
                    nc.gpsimd.partition_broadcast(bc[0:64, :], sums_sb[0:1, row, :])
                    # the broadcast op can only write at base partition 0, so
                    # the odd head's row goes through a base-0 temp + copy-up
                    tm = pn.tile([64, 512], f32, name="tm", tag="tm")
                    nc.gpsimd.partition_broadcast(tm[:], sums_sb[0:1, row + 1, :])
                    nc.vector.tensor_copy(bc[64:P, :], tm[:])
                    nc.vector.reciprocal_approx_fast(bc[:], bc[:])
                    sl = ctxT_sb[:, pr, qs]
                    nc.vector.tensor_mul(sl, sl, bc[:])

            def outproj_rs(qc, sts, rs_rows):
                # output projection for token tiles 4*qc+sts, then
                # reduce-scatter rows `rs_rows` of this chunk's partial with
                # the pair core (ordered after the tiles covering those rows).
                for st in sts:
                    tt = 4 * qc + st
                    ts_ = slice(128 * tt, 128 * (tt + 1))
                    td = slice(128 * st, 128 * (st + 1))
                    for nch in range(2):
                        ns = slice(512 * nch, 512 * (nch + 1))
                        ps = pp.tile([P, 512], f32, name="ps_o", tag="ps")
                        for rr in range(NR):
                            nc.tensor.matmul(
                                ps[:],
                                ctxT_sb[:, rr, ts_],
                                wo_sb[:, rr, ns],
                                start=(rr == 0),
                                stop=(rr == NR - 1),
                            )
                        ot = po_sb.tile([P, 512], bf16, name="ot", tag="ot")
                        nc.vector.tensor_add(ot[:], ps[:], bo_sb[:, ns])
                        nc.sync.dma_start(partial[qc][td, ns], ot[:])
                import concourse.mybir as mybir

                a, b = rs_rows
                nc.gpsimd.collective_compute(
                    "ReduceScatter",
                    mybir.AluOpType.add,
                    replica_groups=[[0, 1], [2, 3], [4, 5], [6, 7]],
                    ins=[partial[qc][a:b, :]],
                    outs=[rs_out[qc][a // 2 : b // 2, :]],
                )

            project(0)
            project(1)
            nc.sync.dma_start(wo_sb[:], wo.rearrange("(ko p) f -> p ko f", p=P))
            nc.sync.dma_start(bo_sb[:], bo_b[:])
            attend(0, [0, 1])
            norm(0, [0, 1])
            attend(0, [2, 3])
            norm(0, [2, 3])
            project(2)
            outproj_rs(0, [0, 1, 2, 3], (0, 512))
            attend(1, [0, 1])
            norm(1, [0, 1])
            attend(1, [2, 3])
            norm(1, [2, 3])
            project(3)
            outproj_rs(1, [0, 1, 2, 3], (0, 512))
            attend(2, [0, 1])
            norm(2, [0, 1])
            attend(2, [2, 3])
            norm(2, [2, 3])
            attend(3, [0, 1])
            outproj_rs(2, [0, 1, 2, 3], (0, 512))
            norm(3, [0, 1])
            attend(3, [2])
            norm(3, [2])
            attend(3, [3])
            norm(3, [3])
            outproj_rs(3, [0, 1, 2, 3], (0, 512))
            # rs_out -> out copies all sit at the end so their waits on the
            # collectives never head-of-line-block the DMA queue mid-kernel
            for qc in range(4):
                nc.sync.dma_start(
                    out_ext[256 * qc : 256 * (qc + 1), :], rs_out[qc][:]
                )

    nc.compile()
    return nc


def _in_maps(x, Wq, Wk, Wv, Wo, bo):
    import ml_dtypes

    bf16 = ml_dtypes.bfloat16
    masks = _build_masks().astype(bf16)
    maps = []
    for c in range(8):
        b, g = c // 2, c % 2
        cols = slice(DG * g, DG * (g + 1))
        maps.append(
            {
                "xT": np.ascontiguousarray(np.asarray(x)[b].T).astype(bf16),
                "wq": np.ascontiguousarray(np.asarray(Wq)[:, cols]).astype(bf16),
                "wk": np.ascontiguousarray(np.asarray(Wk)[:, cols]).astype(bf16),
                "wv": np.ascontiguousarray(np.asarray(Wv)[:, cols]).astype(bf16),
                "wo": np.ascontiguousarray(np.asarray(Wo)[cols, :]).astype(bf16),
                "bo_b": np.broadcast_to(
                    np.asarray(bo, dtype=np.float32) / G, (P, D)
                ).copy(),
                "masks": masks,
            }
        )
    return maps


def _get_nc():
    if "nc" not in _CACHE:
        _CACHE["nc"] = _build_bass()
    return _CACHE["nc"]


def run(inputs, trace=False):
    from concourse.bass_utils import run_bass_kernel_spmd

    nc = _get_nc()
    maps = _in_maps(**inputs)
    res = run_bass_kernel_spmd(nc, maps, list(range(8)), trace=trace)
    out = np.empty((B, S, D), dtype=np.float32)
    # reduce-scatter chunks as issued by the kernel: (token0, rows_in)
    chunks = [(0, 512), (512, 512), (1024, 512), (1536, 512)]
    for c in range(8):
        b, g = c // 2, c % 2
        r = np.asarray(res.results[c]["out"]).astype(np.float32)
        off = 0
        for tok0, L in chunks:
            h = L // 2
            out[b, tok0 + h * g : tok0 + h * (g + 1), :] = r[off : off + h]
            off += h
    return out, res


def kernel(x, Wq, Wk, Wv, Wo, bo):
    out, _ = run(dict(x=x, Wq=Wq, Wk=Wk, Wv=Wv, Wo=Wo, bo=bo))
    return out
